# revision 1
# baseline (speedup 1.0000x reference)
"""Trainium2 Bass kernel for DCTTransform (2D DCT -> 4 freq masks -> IDCT), v2.

Data parallel: 96 images of 512x512 across 8 cores (12 each).

Even/odd DCT symmetry (D[f, N-1-s] = (-1)^f D[f, s]) is folded into ALL four
matmul stages:
  * forward (M1/M2) as in v1, but Y is kept in PACKED frequency layout
    (f1 blocks e0,o0,e1,o1 on partitions; f2 as [even 256 | odd 256] free),
    which makes every anti-triangular mask a clean tri/tri' multiply per
    parity pane and exposes the parity structure the inverse folds need.
  * inverse stage 1 (M3) computes Vn[f2,s1<256] and Vr[f2,511-s1] directly
    in the PE: Vn groups contract even-f1 blocks against D_even rows plus
    odd-f1 blocks against D_odd rows; Vr uses a NEGATED D_odd, so no DVE
    combine is needed (PSUM-reading tensor_tensor measures ~2 cyc/elem,
    double a plain copy, so PE-side signs beat DVE-side combines).
  * inverse stage 2 (M4) is D-stationary and s2-folded: 16 matmuls/image
    with constant weights produce E4/O4 = even/odd-f2 partial sums in
    [s2-block, s1] orientation; the final out[s2'] = E4+O4,
    out[511-s2'] = E4-O4 combine happens ON HOST after DMA (pure fp16
    copies PSUM->SBUF on-chip, no TT).

mask3 == ones -> LL == x, returned on host. Engine split: DVE does input
folds/flips + m1 fold-combines + mask multiplies; scalar drains Y/V/out
PSUM tiles; outputs are fp16, upcast+assembled on host.
"""

import sys

if "/opt/trn_rl_repo" not in sys.path:
    sys.path.insert(0, "/opt/trn_rl_repo")

import numpy as np

NCORES = 8
IMG = 512
P = 128
H = 256


def build_program(nimg):
    import concourse.bacc as bacc
    import concourse.tile as tile
    import concourse.mybir as mybir

    f32, f16 = mybir.dt.float32, mybir.dt.float16

    nc = bacc.Bacc("TRN2", target_bir_lowering=False, debug=False, num_devices=NCORES)

    xar_d = nc.dram_tensor("xar", [nimg, P, 4, IMG], f16, kind="ExternalInput")
    # packed f16 consts: ce|co|de|do|don (512 cols each) + dsn|dsr (256) +
    # trix (384) + t64 (192, on all 128 partitions, rows 64:128 zero)
    CW0 = 2 * 512                    # hottest: ce co (gates M1 of image 0)
    CW = 3 * 512 + 2 * 256           # warm: de do don dsn dsr
    CW2 = 384 + 192                   # cold: trix t64
    cst0_d = nc.dram_tensor("cst0", [P, CW0], f16, kind="ExternalInput")
    cst_d = nc.dram_tensor("cst", [P, CW], f16, kind="ExternalInput")
    cst2_d = nc.dram_tensor("cst2", [P, CW2], f16, kind="ExternalInput")
    trixf_d = nc.dram_tensor("trixf", [P, 384], f32, kind="ExternalInput")
    out_d = {
        nm: nc.dram_tensor(nm, [nimg, P, 2, 2, IMG], f16, kind="ExternalOutput")
        for nm in ("lh", "hl", "hh")
    }

    def eo(ap3, lo, hi):
        # y[:, blk, :] view -> [p, 2, hi-lo]: cols {lo:hi} of even half + odd half
        return ap3.rearrange("p (g c) -> p g c", g=2)[:, :, lo:hi]

    with tile.TileContext(nc) as tc:
        with (
            tc.tile_pool(name="const", bufs=1) as cpool,
            tc.tile_pool(name="io", bufs=4) as iopool,
            tc.tile_pool(name="work", bufs=2) as wpool,
            tc.tile_pool(name="psA", bufs=4, space="PSUM") as psA,
            tc.tile_pool(name="psB", bufs=4, space="PSUM") as psB,
        ):
            cst0 = cpool.tile([P, CW0], f16, tag="cst0")
            cst = cpool.tile([P, CW], f16, tag="cst")
            cst2 = cpool.tile([P, CW2], f16, tag="cst2")
            trixf = cpool.tile([P, 384], f32, tag="trixf")
            nc.sync.dma_start(cst0[:], cst0_d[:])

            def cview(lo, w, k=None):
                v = cst[:, lo : lo + w]
                return v.rearrange("p (k h) -> p k h", k=k) if k else v
            ce = cst0[:, 0:512].rearrange("p (k h) -> p k h", k=2)
            co = cst0[:, 512:1024].rearrange("p (k h) -> p k h", k=2)
            de = cview(0, 512, 2)
            do = cview(512, 512, 2)
            don = cview(1024, 512, 2)
            dsn = cview(1536, 256)
            dsr = cview(1792, 256)
            def c2view(lo, w, k=None):
                v = cst2[:, lo : lo + w]
                return v.rearrange("p (k h) -> p k h", k=k) if k else v
            trix = c2view(0, 384)
            t64 = cst2[0:64, 384 : 384 + 192]

            tri2 = eo(trix[:, 0:256], 0, 128)      # (tri | tri)   [p,2,128]
            tri2f = eo(trixf[:, 0:256], 0, 128)
            trip2f = eo(trixf[:, 128:384], 0, 128)
            trip2 = eo(trix[:, 128:384], 0, 128)   # (tri | trip)
            t642 = eo(t64[:, 0:128], 0, 64)        # (t64 | t64)   [64p,2,64]
            t64p2 = eo(t64[:, 64:192], 0, 64)      # (t64 | t64p)

            def stage_in(img):
                # input DMA + folds (gpsimd: emitted 2 images ahead, so its
                # latency is slack) + flips (DVE)
                xar = iopool.tile([P, 4, IMG], f16, tag="xar", name="xar")
                nc.sync.dma_start(xar[:], xar_d[img])
                xa = xar[:, 0:2, :]
                xr = xar[:, 2:4, :]
                xp = iopool.tile([P, 2, IMG], f16, tag="xp", name="xp")
                xm = iopool.tile([P, 2, IMG], f16, tag="xm", name="xm")
                nc.vector.tensor_add(xp[:], xa, xr)
                nc.vector.tensor_sub(xm[:], xa, xr)
                xcp = iopool.tile([P, 2, IMG], f16, tag="xcp", name="xcp")
                xcm = iopool.tile([P, 2, IMG], f16, tag="xcm", name="xcm")
                nc.vector.tensor_add(xcp[:], xa[:, :, ::-1], xr[:, :, ::-1])
                nc.vector.tensor_sub(xcm[:], xa[:, :, ::-1], xr[:, :, ::-1])
                return xp, xm, xcp, xcm

            def stage_front(img, ins):
                # M1 + fold-combine; returns (m1p, m1m)
                xp, xm, xcp, xcm = ins

                # M1: pn/pr[mp] = [128, 512] (f1-even 0:256 | f1-odd 256:512)
                # Groups alternate tiles so consecutive PE groups never share
                # a PSUM bank (keeps LDWEIGHTS hidden).
                pn = [psA.tile([P, IMG], f32, tag="a", name=f"pn{i}") for i in range(2)]
                pr = [psA.tile([P, IMG], f32, tag="a", name=f"pr{i}") for i in range(2)]
                for mp in range(2):
                    for par, (src_n, src_r, rhs) in enumerate(
                        ((xp, xcp, ce), (xm, xcm, co))
                    ):
                        s = 256 * par
                        for t, src in ((pn[mp], src_n), (pr[mp], src_r)):
                            for k in range(2):
                                nc.tensor.matmul(
                                    t[:, s : s + 256], src[:, k, P * mp : P * (mp + 1)],
                                    rhs[:, k, :], start=(k == 0), stop=(k == 1))

                # M1 fold combine via fp16 SBUF (PSUM-TT is 2 cyc/elem;
                # copy-then-fp16-TT is ~2x cheaper and frees PSUM fast)
                m1n = wpool.tile([P, 2, IMG], f16, tag="m1n", name="m1n")
                m1r = wpool.tile([P, 2, IMG], f16, tag="m1r", name="m1r")
                for mp in range(2):
                    nc.scalar.copy(m1n[:, mp, :], pn[mp][:])
                    nc.vector.tensor_copy(m1r[:, mp, :], pr[mp][:])
                m1p = wpool.tile([P, 2, IMG], f16, tag="m1p", name="m1p")
                m1m = wpool.tile([P, 2, IMG], f16, tag="m1m", name="m1m")
                nc.vector.tensor_add(m1p[:], m1n[:], m1r[:])
                nc.vector.tensor_sub(m1m[:], m1n[:], m1r[:])
                return m1p, m1m

            ins = {0: stage_in(0)}
            if nimg > 1:
                ins[1] = stage_in(1)
            nc.sync.dma_start(cst[:], cst_d[:])
            nc.sync.dma_start(cst2[:], cst2_d[:])
            front = stage_front(0, ins.pop(0))
            nc.sync.dma_start(trixf[:], trixf_d[:])
            for img in range(nimg):
                m1p, m1m = front
                if img + 2 < nimg:
                    ins[img + 2] = stage_in(img + 2)
                if img + 1 < nimg:
                    front = stage_front(img + 1, ins.pop(img + 1))

                # ---- M2: y [128, 2(F: e0,o0), 512(f2: e|o)]; e1/o1 stay in
                # PSUM (their unmasked values are never consumed) and are
                # masked straight into tmA1 tiles below.
                y = wpool.tile([P, 2, IMG], f16, tag="y")
                FCR = [0, 256, 128, 384]  # col starts in m1p f1-packed axis
                tmA1 = {}
                for pair in (0, 2):
                    ts = [psA.tile([P, IMG], f32, tag="a", name=f"py{pair}_{i}") for i in range(2)]
                    for par, (m1, rhs) in enumerate(((m1p, ce), (m1m, co))):
                        s = 256 * par
                        for j, t in enumerate(ts):
                            c0 = FCR[pair + j]
                            for k in range(2):
                                nc.tensor.matmul(
                                    t[:, s : s + 256], m1[:, k, c0 : c0 + P],
                                    rhs[:, k, :], start=(k == 0), stop=(k == 1))
                    if pair == 0:
                        for j, t in enumerate(ts):
                            nc.scalar.copy(y[:, j, :], t[:])
                    else:
                        for j, (t, msk) in enumerate(zip(ts, (tri2f, trip2f))):
                            tm = wpool.tile([P, 256], f16, tag=f"tmA1{j}", name=f"tmA1{j}")
                            nc.vector.tensor_mul(
                                tm[:].rearrange("p (g c) -> p g c", g=2),
                                t[:].rearrange("p (g c) -> p g c", g=2)[:, :, 0:128], msk)
                            tmA1[j] = tm

                # ---- masked tiles (fp16). tmLH stacks the even/odd-f1 LH
                # halves on partitions 0:64 / 64:128 (via SBUF->SBUF DMA) so
                # M3-LH runs one full-depth matmul per sign.
                tmLH = wpool.tile([P, P], f16, tag="tmLH")
                tmLH_o = wpool.tile([64, P], f16, tag="tmLHo")
                nc.vector.tensor_mul(tmLH[0:64, :].rearrange("p (g c) -> p g c", g=2),
                                     eo(y[0:64, 0, :], 0, 64), t642)
                nc.vector.tensor_mul(tmLH_o[:].rearrange("p (g c) -> p g c", g=2),
                                     eo(y[0:64, 1, :], 0, 64), t64p2)
                nc.sync.dma_start(tmLH[64:128, :], tmLH_o[:])
                tms = {}
                for nm, blk, lo, msk, eng in (
                    ("HLe", 0, 0, tri2, nc.vector), ("HLo", 1, 0, trip2, nc.vector),
                    ("B1e", 0, 128, tri2, nc.gpsimd), ("B1o", 1, 128, trip2, nc.gpsimd),
                ):
                    t = wpool.tile([P, 256], f16, tag=f"tm{nm}")
                    eng.tensor_mul(t[:].rearrange("p (g c) -> p g c", g=2),
                                   eo(y[:, blk, :], lo, lo + 128), msk)
                    tms[nm] = t
                tms["A1e"], tms["A1o"] = tmA1[0], tmA1[1]

                # ---- M3 direct: V tiles, fp16, cols = (Vn 0:256 | Vr 256:512)
                # Groups interleaved across tiles; Vn uses do, Vr uses -do.
                # LH: stacked full-depth: rhs dsn = [D_even;D_odd] rows, dsr neg-odd
                vLH = wpool.tile([P, IMG], f16, tag="vLH")
                tLH = psB.tile([P, IMG], f32, tag="b", name="tLH")
                vHL = wpool.tile([P, 2, IMG], f16, tag="vHL")
                tHL = [psB.tile([P, IMG], f32, tag="b", name=f"tHL{i}") for i in range(2)]
                for half, rhs_o, dstk in ((0, do, dsn), (1, don, dsr)):
                    s = 256 * half
                    nc.tensor.matmul(tLH[:, s : s + 256], tmLH[:, :],
                                     dstk, start=True, stop=True)
                    for par in range(2):
                        c0 = P * par
                        nc.tensor.matmul(tHL[par][:, s : s + 256], tms["HLe"][:, c0 : c0 + P],
                                         de[:, 0, :], start=True, stop=False)
                        nc.tensor.matmul(tHL[par][:, s : s + 256], tms["HLo"][:, c0 : c0 + P],
                                         rhs_o[:, 0, :], start=False, stop=True)
                nc.scalar.copy(vLH[:], tLH[:])
                for par in range(2):
                    nc.scalar.copy(vHL[:, par, :], tHL[par][:])

                # HH: vHH0 (J=E0|O0: f2 b<128), vHH1 (J=E1|O1: b in [128,256))
                vHH0 = wpool.tile([P, 2, IMG], f16, tag="vHH0")
                vHH1 = wpool.tile([P, 2, IMG], f16, tag="vHH1")
                tH0 = [psB.tile([P, IMG], f32, tag="b", name=f"tH0{i}") for i in range(2)]
                tH1 = [psB.tile([P, IMG], f32, tag="b", name=f"tH1{i}") for i in range(2)]
                for half, rhs_o in ((0, do), (1, don)):  # Vn, Vr
                    s = 256 * half
                    for par in range(2):  # J parity: even-f2 cols / odd-f2 cols
                        yc0 = 256 * par
                        nc.tensor.matmul(tH0[par][:, s : s + 256], y[:, 0, yc0 : yc0 + P],
                                         de[:, 0, :], start=True, stop=False)
                        nc.tensor.matmul(tH0[par][:, s : s + 256], y[:, 1, yc0 : yc0 + P],
                                         rhs_o[:, 0, :], start=False, stop=False)
                        nc.tensor.matmul(tH0[par][:, s : s + 256], tms["A1e"][:, P * par : P * (par + 1)],
                                         de[:, 1, :], start=False, stop=False)
                        nc.tensor.matmul(tH0[par][:, s : s + 256], tms["A1o"][:, P * par : P * (par + 1)],
                                         rhs_o[:, 1, :], start=False, stop=True)
                    for par in range(2):
                        nc.tensor.matmul(tH1[par][:, s : s + 256], tms["B1e"][:, P * par : P * (par + 1)],
                                         de[:, 0, :], start=True, stop=False)
                        nc.tensor.matmul(tH1[par][:, s : s + 256], tms["B1o"][:, P * par : P * (par + 1)],
                                         rhs_o[:, 0, :], start=False, stop=True)
                for par in range(2):
                    nc.vector.tensor_copy(vHH0[:, par, :], tH0[par][:])
                for par in range(2):
                    (nc.scalar.copy if par == 0 else nc.vector.tensor_copy)(
                        vHH1[:, par, :], tH1[par][:])

                # ---- M4 D-stationary folded: per mask, per m2: E4/O4 [128, 512]
                for nm, spec in (
                    ("lh", "LH"), ("hl", "HL"), ("hh", "HH"),
                ):
                    ot = iopool.tile([P, 2, 2, IMG], f16, tag=f"ot_{nm}")
                    for m2 in range(2):
                        c0 = P * m2
                        tE = psB.tile([P, IMG], f32, tag="b")
                        tO = psB.tile([P, IMG], f32, tag="b")
                        if spec == "LH":
                            # stacked: slots are out_n / out_r directly (no host +/-)
                            nc.tensor.matmul(tE[:, :], dsn[:, c0 : c0 + P],
                                             vLH[:, :], start=True, stop=True)
                            nc.tensor.matmul(tO[:, :], dsr[:, c0 : c0 + P],
                                             vLH[:, :], start=True, stop=True)
                        elif spec == "HL":
                            nc.tensor.matmul(tE[:, :], de[:, 0, c0 : c0 + P],
                                             vHL[:, 0, :], start=True, stop=True)
                            nc.tensor.matmul(tO[:, :], do[:, 0, c0 : c0 + P],
                                             vHL[:, 1, :], start=True, stop=True)
                        else:
                            nc.tensor.matmul(tE[:, :], de[:, 0, c0 : c0 + P],
                                             vHH0[:, 0, :], start=True, stop=False)
                            nc.tensor.matmul(tE[:, :], de[:, 1, c0 : c0 + P],
                                             vHH1[:, 0, :], start=False, stop=True)
                            nc.tensor.matmul(tO[:, :], do[:, 0, c0 : c0 + P],
                                             vHH0[:, 1, :], start=True, stop=False)
                            nc.tensor.matmul(tO[:, :], do[:, 1, c0 : c0 + P],
                                             vHH1[:, 1, :], start=False, stop=True)
                        nc.scalar.copy(ot[:, m2, 0, :], tE[:])
                        if m2 == 1:
                            nc.vector.tensor_copy(ot[:, m2, 1, :], tO[:])
                        else:
                            nc.scalar.copy(ot[:, m2, 1, :], tO[:])
                        if img == nimg - 1:
                            nc.sync.dma_start(out_d[nm][img, :, m2], ot[:, m2])
                    if img < nimg - 1:
                        nc.sync.dma_start(out_d[nm][img], ot[:])

    nc.compile()
    return nc


_prog_cache = {}

TRACE = False
TRACE_KWARGS = {}
LAST_RESULTS = None


def _get_prog(nimg):
    if nimg not in _prog_cache:
        _prog_cache[nimg] = build_program(nimg)
    return _prog_cache[nimg]


def _dct_f64():
    k = np.arange(IMG, dtype=np.float64)[:, None]
    m = np.arange(IMG, dtype=np.float64)[None, :]
    D = np.cos(np.pi * (2.0 * m + 1.0) * k / (2.0 * IMG)) * np.sqrt(2.0 / IMG)
    D[0] *= 1.0 / np.sqrt(2.0)
    return D


def _consts_f16():
    D = _dct_f64()
    ce = D[0::2, 0:H].T.reshape(2, P, H).transpose(1, 0, 2)
    co = D[1::2, 0:H].T.reshape(2, P, H).transpose(1, 0, 2)
    de = D[0::2, 0:H].reshape(2, P, H).transpose(1, 0, 2)
    do = D[1::2, 0:H].reshape(2, P, H).transpose(1, 0, 2)
    ii = np.arange(P)[:, None]
    jj = np.arange(P)[None, :]
    tri = (ii + jj <= 127).astype(np.float64)
    trip = (ii + jj <= 126).astype(np.float64)
    i4 = np.arange(64)[:, None]
    j4 = np.arange(64)[None, :]
    t64 = (i4 + j4 <= 63).astype(np.float64)
    t64p = (i4 + j4 <= 62).astype(np.float64)
    # dsn/dsr: stacked [D[2a, s<256] a<64 ; +/-D[2a+1, s<256]] for the LH path
    dsn = np.concatenate([D[0:128:2, 0:H], D[1:128:2, 0:H]], 0)
    dsr = np.concatenate([D[0:128:2, 0:H], -D[1:128:2, 0:H]], 0)
    trix = np.concatenate([tri, tri, trip], 1)
    t64w = np.zeros((P, 192))
    t64w[0:64] = np.concatenate([t64, t64, t64p], 1)
    cst0 = np.concatenate([ce.reshape(P, 512), co.reshape(P, 512)], axis=1)
    cst = np.concatenate([
        de.reshape(P, 512), do.reshape(P, 512), (-do).reshape(P, 512), dsn, dsr,
    ], axis=1)
    cst2 = np.concatenate([trix, t64w], axis=1)
    return {
        "cst0": np.ascontiguousarray(cst0).astype(np.float16),
        "cst": np.ascontiguousarray(cst).astype(np.float16),
        "cst2": np.ascontiguousarray(cst2).astype(np.float16),
        "trixf": np.ascontiguousarray(trix).astype(np.float32),
    }


def _assemble(arr, direct=False):
    """arr [n, 128, 2(m2), 2(par), 512] f16 -> out [n, 512, 512] f32.

    direct=False: par slots are E4/O4 partial sums -> out pair = E+O / E-O.
    direct=True (LH): par slots are already out_n / out_r.
    """
    a = arr.astype(np.float32)
    A = a[:, :, :, 0, :].transpose(0, 2, 1, 3).reshape(-1, 256, 512)  # [n, s2q, c]
    B = a[:, :, :, 1, :].transpose(0, 2, 1, 3).reshape(-1, 256, 512)
    if direct:
        Pl, Mn = A, B
    else:
        Pl = A + B
        Mn = A - B

    def to_s1(t):
        return np.concatenate([t[:, :, 0:256], t[:, :, 256:512][:, :, ::-1]], 2)

    n = arr.shape[0]
    out = np.empty((n, IMG, IMG), dtype=np.float32)
    out[:, :, 0:256] = to_s1(Pl).transpose(0, 2, 1)
    out[:, :, 256:512] = to_s1(Mn).transpose(0, 2, 1)[:, :, ::-1]
    return out


def kernel(x, masks):
    from concourse.bass_utils import run_bass_kernel_spmd

    x = np.ascontiguousarray(np.asarray(x), dtype=np.float32)
    B, C, Hh, W = x.shape
    n = B * C
    per = n // NCORES
    x16 = x.reshape(n, Hh, W).astype(np.float16)

    xa = x16[:, 0:H, :].reshape(n, 2, P, W).transpose(0, 2, 1, 3)
    xr = x16[:, ::-1, :][:, 0:H, :].reshape(n, 2, P, W).transpose(0, 2, 1, 3)
    xar = np.ascontiguousarray(np.concatenate([xa, xr], axis=2))
    consts = _consts_f16()

    in_maps = [
        {"xar": xar[c * per : (c + 1) * per], **consts}
        for c in range(NCORES)
    ]

    nc = _get_prog(per)
    res = run_bass_kernel_spmd(
        nc, in_maps, list(range(NCORES)), trace=TRACE, **TRACE_KWARGS
    )
    global LAST_RESULTS
    LAST_RESULTS = res

    outs = {}
    for nm in ("lh", "hl", "hh"):
        raw = np.concatenate([res.results[c][nm] for c in range(NCORES)], axis=0)
        outs[nm] = _assemble(raw, direct=(nm == "lh")).reshape(B, C, Hh, W)
    LL = x.copy()
    return (LL, outs["lh"], outs["hl"], outs["hh"])

